# revision 1
# baseline (speedup 1.0000x reference)
"""Trainium2 Bass kernel for the segment-reduce cosine loss problem.

Reference computation (per sample b, S=32 labels):
  onehot[l,s] = (attributes[b,l] == s+1)
  seg_sum[s,:] = sum_l onehot[l,s] * text_feats[b,l,:]
  seg_mean     = seg_sum / count[s]
  cos[s] = <Vgs[b,s], seg_mean[s]> / max(|Vgs[b,s]| * |seg_mean[s]|, 1e-8)
  loss = mean_b (1 - mean_s cos[b,s]) = 1 - (sum_{b,s} cos) / (B*S)

Sharding: pure data parallel over batch. Each of the 8 cores processes 8
samples and outputs its [S, 8] cos matrix; the host sums them into the
scalar loss. Cosine similarity is invariant to positive scaling of
seg_mean, so the kernel works with seg_sum directly and never computes
the counts (the 1e-8 clamp is unreachable for this data distribution
either way: |V|*|seg_sum| is O(1e3)).

Per-core kernel (one NeuronCore, Tile framework on bacc):
  - attributes are cast to f32 and PE-transposed so each token position
    lands on a partition; all 8 onehot blocks [128, 32] for a sample are
    built in one DVE is_equal against an iota row (stride-0 broadcasts).
  - seg_sum runs on the PE in float32r (full-rate fp32 mode, tf32-like
    precision; the final scalar averages the noise away): lhsT = onehot
    chunk (stationary), rhs = text chunk [128, 512], accumulated over the
    8 L-chunks into PSUM [32, 512] x 2. Text streams in per-chunk 512 KB
    DMAs (24 tile buffers deep) and is the critical path: ~32 MB/core.
  - Vgs loads and |Vg|^2 norms (ACT Square with fused accum) are hoisted
    ahead of the text stream; per-sample epilogue computes <ss, Vg> (DVE
    mult from PSUM + reduce) and |ss|^2 (ACT Square + accum from PSUM).
  - cosine assembly (mult, sqrt, eps-clamp, reciprocal) is batched over
    all samples at [32, 8]; the Sqrt ACT table is pre-loaded at kernel
    start so the load is off the tail.
"""

import numpy as np

import concourse.mybir as mybir
import concourse.tile as tile
from concourse import bacc
from concourse.bass_utils import run_bass_kernel_spmd

B, L, D, S = 64, 1024, 1024, 32
N_CORES = 8
BPC = B // N_CORES        # samples per core
NCHUNK = L // 128         # L-chunks of 128 positions
EPS = 1e-8

F32 = mybir.dt.float32
F32R = mybir.dt.float32r
I32 = mybir.dt.int32
ALU = mybir.AluOpType
AXIS = mybir.AxisListType
ACTF = mybir.ActivationFunctionType


def build_bass():
    nc = bacc.Bacc(
        "TRN2", target_bir_lowering=False, debug=False, num_devices=N_CORES
    )
    attrs_d = nc.dram_tensor("attributes", [BPC, L], I32, kind="ExternalInput")
    text_d = nc.dram_tensor("text_feats", [BPC, L, D], F32R, kind="ExternalInput")
    vgs_d = nc.dram_tensor("Vgs", [BPC, S, D], F32, kind="ExternalInput")
    out_d = nc.dram_tensor("out", [S, BPC], F32, kind="ExternalOutput")

    with tile.TileContext(nc) as tc:
        with (
            tc.tile_pool(name="const", bufs=1) as const_pool,
            tc.tile_pool(name="text", bufs=24) as text_pool,
            tc.tile_pool(name="oh", bufs=4) as oh_pool,
            tc.tile_pool(name="work", bufs=2) as work_pool,
            tc.tile_pool(name="vgsp", bufs=BPC) as vgs_pool,
            tc.tile_pool(name="small", bufs=2) as small_pool,
            tc.tile_pool(name="psum", bufs=3, space="PSUM") as psum_pool,
            tc.tile_pool(name="psum1", bufs=1, space="PSUM") as psum1_pool,
        ):
            # ---- constants ----
            iota_s = const_pool.tile([128, S], F32, name="iota_s")
            nc.gpsimd.iota(
                iota_s[:], pattern=[[1, S]], base=1, channel_multiplier=0,
                allow_small_or_imprecise_dtypes=True,
            )
            warm = const_pool.tile([128, 1], F32, name="warm")
            nc.vector.memset(warm[:], 1.0)
            nc.scalar.sqrt(warm[:], warm[:])
            # 8x8 identity for the PE transpose of the attribute block
            idrow = const_pool.tile([BPC, BPC], F32, name="idrow")
            nc.gpsimd.iota(
                idrow[:], pattern=[[1, BPC]], base=0, channel_multiplier=0,
                allow_small_or_imprecise_dtypes=True,
            )
            idcol = const_pool.tile([BPC, 1], F32, name="idcol")
            nc.gpsimd.iota(
                idcol[:], pattern=[[0, 1]], base=0, channel_multiplier=1,
                allow_small_or_imprecise_dtypes=True,
            )
            ident = const_pool.tile([BPC, BPC], F32, name="ident")
            nc.vector.tensor_tensor(
                ident[:], idcol[:, 0:1].broadcast_to([BPC, BPC]), idrow[:],
                op=ALU.is_equal,
            )

            # ---- attribute prep: [BPC, L] i32 -> f32 -> transpose to [128, BPC*NCHUNK]
            attr_i = const_pool.tile([BPC, L], I32, name="attr_i")
            nc.scalar.dma_start(attr_i[:], attrs_d[:])
            attr_f = const_pool.tile([BPC, L], F32, name="attr_f")
            nc.vector.tensor_copy(attr_f[:], attr_i[:])
            psum_attr = psum1_pool.tile([128, NCHUNK * BPC], F32, tag="misc", name="psum_attr")
            for c in range(NCHUNK):
                # out[p, b] = attr_f[b, c*128 + p]
                nc.tensor.transpose(
                    psum_attr[:, c * BPC:(c + 1) * BPC],
                    attr_f[:, c * 128:(c + 1) * 128],
                    ident[:],
                )
            # permute (c, b) -> (b, c) while copying out of PSUM, so each
            # sample's NCHUNK attribute scalars are contiguous
            attr_sb = const_pool.tile([128, BPC * NCHUNK], F32, name="attr_sb")
            nc.vector.tensor_copy(
                attr_sb[:].rearrange("p (b c) -> p c b", c=NCHUNK),
                psum_attr[:].rearrange("p (c b) -> p c b", b=BPC),
            )

            # cos values per (attribute s = partition, sample b = column).
            # cos is scale-invariant in seg_mean, so seg_sum is used directly
            # and the 1/cnt normalization is skipped entirely.
            cos_all = const_pool.tile([32, BPC], F32, name="cos_all")
            num_all = const_pool.tile([S, BPC], F32, name="num_all")
            ns_parts = const_pool.tile([S, 2 * BPC], F32, name="ns_parts")

            # hoist all Vgs loads and |Vg|^2 norms ahead of the text stream
            nv_all = const_pool.tile([S, BPC], F32, name="nv_all")
            vgs_tiles = []
            for b in range(BPC):
                vg = vgs_pool.tile([S, D], F32, tag="vg", name=f"vg_{b}")
                nc.scalar.dma_start(vg[:], vgs_d[b])
                vgs_tiles.append(vg)
                sq3 = work_pool.tile([S, D], F32, tag="sq3", name=f"sq3_{b}")
                nc.scalar.activation(
                    sq3[:], vg[:], ACTF.Square, accum_out=nv_all[:, b:b + 1]
                )

            for b in range(BPC):
                # all NCHUNK onehot blocks for this sample in one DVE op:
                # oh_all[p, c, s] = (attr[b, c*128+p] == s+1)
                oh_all = oh_pool.tile([128, NCHUNK * S], F32R, tag="oh", name=f"oh_{b}")
                nc.vector.tensor_tensor(
                    oh_all[:].rearrange("p (c s) -> p c s", s=S),
                    attr_sb[:, b * NCHUNK:(b + 1) * NCHUNK]
                    .unsqueeze(2).broadcast_to([128, NCHUNK, S]),
                    iota_s[:].unsqueeze(1).broadcast_to([128, NCHUNK, S]),
                    op=ALU.is_equal,
                )
                psum_s0 = psum_pool.tile([32, 512], F32, tag="s0", name=f"ps0_{b}")
                psum_s1 = psum_pool.tile([32, 512], F32, tag="s1", name=f"ps1_{b}")
                for c in range(NCHUNK):
                    txc = text_pool.tile([128, D], F32R, tag="tx", name=f"tx_{b}_{c}")
                    rows = text_d[b, c * 128:(c + 1) * 128, :]
                    ohr = oh_all[:, c * S:(c + 1) * S]
                    st, sp = c == 0, c == NCHUNK - 1
                    if b == BPC - 1:
                        # last sample: split each chunk's DMA by D-half so the
                        # matmul on the first half hides its semaphore latency
                        # under the second half's transfer (shorter tail)
                        nc.sync.dma_start(txc[:, 0:512], rows[:, 0:512])
                        nc.sync.dma_start(txc[:, 512:D], rows[:, 512:D])
                    else:
                        nc.sync.dma_start(txc[:], rows)
                    nc.tensor.matmul(
                        psum_s0[:], ohr, txc[:, 0:512], start=st, stop=sp,
                    )
                    nc.tensor.matmul(
                        psum_s1[:], ohr, txc[:, 512:D], start=st, stop=sp,
                    )

                # ---- per-sample epilogue on partitions 0..31 ----
                vg = vgs_tiles[b]
                scr = work_pool.tile([S, D], F32, tag="scr", name=f"scr_{b}")
                sq2 = work_pool.tile([S, D], F32, tag="sq2", name=f"sq2_{b}")
                for h, ps in enumerate((psum_s0, psum_s1)):
                    # seg_sum * Vg (DVE) and seg_sum^2 with fused free-dim
                    # accumulation (ACT), both read straight out of PSUM
                    nc.vector.tensor_tensor(
                        scr[:, h * 512:(h + 1) * 512], ps[:],
                        vg[:, h * 512:(h + 1) * 512], op=ALU.mult,
                    )
                    nc.scalar.activation(
                        sq2[:, h * 512:(h + 1) * 512], ps[:], ACTF.Square,
                        accum_out=ns_parts[:, 2 * b + h:2 * b + h + 1],
                    )

                nc.vector.tensor_reduce(
                    num_all[:, b:b + 1], scr[:], axis=AXIS.X, op=ALU.add
                )

            # ---- batched cosine assembly over all samples [S, BPC] ----
            ns_all = small_pool.tile([S, BPC], F32, name="ns_all")
            nc.vector.tensor_reduce(
                ns_all[:], ns_parts[:].rearrange("s (b h) -> s b h", h=2),
                axis=AXIS.X, op=ALU.add,
            )
            prod = small_pool.tile([S, BPC], F32, name="prod")
            nc.vector.tensor_tensor(prod[:], ns_all[:], nv_all[:], op=ALU.mult)
            sq = small_pool.tile([S, BPC], F32, name="sq")
            nc.scalar.sqrt(sq[:], prod[:])
            den = small_pool.tile([S, BPC], F32, name="den")
            nc.vector.tensor_scalar(
                out=den[:], in0=sq[:], scalar1=float(EPS), scalar2=None,
                op0=ALU.max,
            )
            rec = small_pool.tile([S, BPC], F32, name="rec")
            nc.vector.reciprocal(rec[:], den[:])
            nc.vector.tensor_tensor(cos_all[:], num_all[:], rec[:], op=ALU.mult)

            nc.sync.dma_start(out_d[:], cos_all[:])

    nc.compile()
    return nc


_NC_CACHE = None


def _get_nc():
    global _NC_CACHE
    if _NC_CACHE is None:
        _NC_CACHE = build_bass()
    return _NC_CACHE


def kernel(attributes: np.ndarray, text_feats: np.ndarray, Vgs: np.ndarray) -> np.ndarray:
    assert attributes.shape == (B, L) and attributes.dtype == np.int32
    assert text_feats.shape == (B, L, D)
    assert Vgs.shape == (B, S, D)
    nc = _get_nc()
    in_maps = [
        {
            "attributes": np.ascontiguousarray(attributes[i * BPC:(i + 1) * BPC]),
            "text_feats": np.ascontiguousarray(text_feats[i * BPC:(i + 1) * BPC], dtype=np.float32),
            "Vgs": np.ascontiguousarray(Vgs[i * BPC:(i + 1) * BPC], dtype=np.float32),
        }
        for i in range(N_CORES)
    ]
    res = run_bass_kernel_spmd(nc, in_maps, core_ids=list(range(N_CORES)))
    total = sum(float(r["out"].sum()) for r in res.results)
    loss = 1.0 - total / (B * S)
    return np.asarray(loss, dtype=np.float32)



# revision 2
# speedup vs baseline: 1.9391x; 1.9391x over previous
"""Trainium2 Bass kernel for the segment-reduce cosine loss problem.

Reference computation (per sample b, S=32 labels):
  onehot[l,s] = (attributes[b,l] == s+1)
  seg_sum[s,:] = sum_l onehot[l,s] * text_feats[b,l,:]
  seg_mean     = seg_sum / count[s]
  cos[s] = <Vgs[b,s], seg_mean[s]> / max(|Vgs[b,s]| * |seg_mean[s]|, 1e-8)
  loss = mean_b (1 - mean_s cos[b,s]) = 1 - (sum_{b,s} cos) / (B*S)

Sharding: pure data parallel over batch; each of the 8 cores handles 8
samples and returns its 256 cos values; the host averages.

Performance design (cost model: DMA 360 GB/s aggregate, PE 2.4 GHz):
  - text_feats is quantized to fp8-e4m3 on the host before upload, cutting
    the dominant HBM stream from 32 MB to 8 MB per core (~23 us at
    360 GB/s).  Cosine is scale-invariant in seg_sum and the loss averages
    2048 cos values, so fp8 noise lands ~1e-4 on the loss (gate is 2e-2).
  - The segment-sum matmul runs in fp8 DoubleRow perf mode: lhsT is a
    text d-tile [128L, 2, 128D] (stationary), rhs the onehot pair
    [128L, 2, 32] (moving), contracting 256 L-positions per instruction at
    0.5 cycles/row -> out ssT [128D, 32S] per d-tile, 16 cycles each.
  - The transposed [D, S] layout puts the epilogue on all 128 partitions:
    prod = ssT*vgT (DVE) and ssT^2 / vgT^2 (ACT) are [128, 256] ops, and
    the D-reductions (num, |ss|^2, |Vg|^2) become a ones-vector matmul
    chain into PSUM [1, 96] per sample.  Vgs is pre-transposed/packed to
    [128, b t s] fp8 on the host so vgT loads in one contiguous DMA.
  - Cosine assembly is batched over all 64 (s,b) pairs at the end.
"""

import numpy as np
import ml_dtypes

import concourse.mybir as mybir
import concourse.tile as tile
from concourse import bacc
from concourse.bass_utils import run_bass_kernel_spmd

B, L, D, S = 64, 1024, 1024, 32
N_CORES = 8
BPC = B // N_CORES        # samples per core
NCHUNK = L // 128         # L-chunks of 128 positions
NPAIR = NCHUNK // 2       # DoubleRow chunk pairs (256 positions each)
NDT = D // 128            # d-tiles of 128 feature columns
EPS = 1e-8

F32 = mybir.dt.float32
F8 = mybir.dt.float8e4
BF16 = mybir.dt.bfloat16
I32 = mybir.dt.int32
ALU = mybir.AluOpType
ACTF = mybir.ActivationFunctionType
PERF = mybir.MatmulPerfMode

NP_F8 = ml_dtypes.float8_e4m3


def build_bass():
    nc = bacc.Bacc(
        "TRN2", target_bir_lowering=False, debug=False, num_devices=N_CORES
    )
    attrs_d = nc.dram_tensor("attributes", [BPC, L], I32, kind="ExternalInput")
    text_d = nc.dram_tensor("text_feats", [BPC, L, D], F8, kind="ExternalInput")
    # host-packed transposed Vgs: vgt[p, ((b*NDT + t)*S + s)] = Vgs[b, s, t*128+p]
    vgt_d = nc.dram_tensor("Vgs", [128, BPC * NDT * S], F8, kind="ExternalInput")
    out_d = nc.dram_tensor("out", [1, BPC * S], F32, kind="ExternalOutput")

    with tile.TileContext(nc) as tc:
        with (
            tc.tile_pool(name="const", bufs=1) as const_pool,
            tc.tile_pool(name="text", bufs=12) as text_pool,
            tc.tile_pool(name="oh", bufs=4) as oh_pool,
            tc.tile_pool(name="sst", bufs=2) as sst_pool,
            tc.tile_pool(name="combo", bufs=BPC) as combo_pool,
            tc.tile_pool(name="small", bufs=2) as small_pool,
            tc.tile_pool(name="psum", bufs=7, space="PSUM") as psum_pool,
            tc.tile_pool(name="psumr", bufs=1, space="PSUM") as psumr_pool,
        ):
            # ---- constants ----
            iota_s = const_pool.tile([128, S], F32, name="iota_s")
            nc.gpsimd.iota(
                iota_s[:], pattern=[[1, S]], base=1, channel_multiplier=0,
                allow_small_or_imprecise_dtypes=True,
            )
            warm = const_pool.tile([128, 1], F32, name="warm")
            nc.vector.memset(warm[:], 1.0)
            nc.scalar.sqrt(warm[:], warm[:])
            ones_bf = const_pool.tile([128, 1], BF16, name="ones_bf")
            nc.vector.memset(ones_bf[:], 1.0)
            # 8x8 identity for the PE transpose of the attribute block
            idrow = const_pool.tile([BPC, BPC], F32, name="idrow")
            nc.gpsimd.iota(
                idrow[:], pattern=[[1, BPC]], base=0, channel_multiplier=0,
                allow_small_or_imprecise_dtypes=True,
            )
            idcol = const_pool.tile([BPC, 1], F32, name="idcol")
            nc.gpsimd.iota(
                idcol[:], pattern=[[0, 1]], base=0, channel_multiplier=1,
                allow_small_or_imprecise_dtypes=True,
            )
            ident = const_pool.tile([BPC, BPC], F32, name="ident")
            nc.vector.tensor_tensor(
                ident[:], idcol[:, 0:1].broadcast_to([BPC, BPC]), idrow[:],
                op=ALU.is_equal,
            )

            # ---- attribute prep: [BPC, L] i32 -> f32 -> transpose to [128, BPC*NCHUNK]
            attr_i = const_pool.tile([BPC, L], I32, name="attr_i")
            nc.scalar.dma_start(attr_i[:], attrs_d[:])
            attr_f = const_pool.tile([BPC, L], F32, name="attr_f")
            nc.vector.tensor_copy(attr_f[:], attr_i[:])
            psum_attr = psum_pool.tile(
                [128, NCHUNK * BPC], F32, tag="ss", name="psum_attr"
            )
            for c in range(NCHUNK):
                # out[p, b] = attr_f[b, c*128 + p]
                nc.tensor.transpose(
                    psum_attr[:, c * BPC:(c + 1) * BPC],
                    attr_f[:, c * 128:(c + 1) * 128],
                    ident[:],
                )
            # permute (c, b) -> (b, c) while copying out of PSUM
            attr_sb = const_pool.tile([128, BPC * NCHUNK], F32, name="attr_sb")
            nc.vector.tensor_copy(
                attr_sb[:].rearrange("p (b c) -> p c b", c=NCHUNK),
                psum_attr[:].rearrange("p (c b) -> p c b", b=BPC),
            )

            # transposed Vgs for all samples: [128, (b t s)] in one DMA
            vgt = const_pool.tile([128, BPC * NDT * S], F8, name="vgt")
            nc.scalar.dma_start(vgt[:], vgt_d[:])

            # combo[b] layout: [128, t, (prod | ss^2 | vg^2)] per sample.
            # vg^2 is filled up-front (overlaps the text stream).
            combos = []
            for b in range(BPC):
                cb = combo_pool.tile([128, NDT, 3 * S], BF16, tag="cb", name=f"cb_{b}")
                nc.scalar.activation(
                    cb[:, :, 2 * S:3 * S],
                    vgt[:, b * NDT * S:(b + 1) * NDT * S]
                    .rearrange("p (t s) -> p t s", s=S),
                    ACTF.Square,
                )
                combos.append(cb)

            asm = const_pool.tile([1, BPC, 3 * S], F32, name="asm")

            for b in range(BPC):
                # all NCHUNK onehot blocks for this sample in one DVE op:
                # oh_all[p, c, s] = (attr[b, c*128+p] == s+1), in fp8
                oh_all = oh_pool.tile([128, NCHUNK * S], F8, tag="oh", name=f"oh_{b}")
                nc.vector.tensor_tensor(
                    oh_all[:].rearrange("p (c s) -> p c s", s=S),
                    attr_sb[:, b * NCHUNK:(b + 1) * NCHUNK]
                    .unsqueeze(2).broadcast_to([128, NCHUNK, S]),
                    iota_s[:].unsqueeze(1).broadcast_to([128, NCHUNK, S]),
                    op=ALU.is_equal,
                )
                oh_v = oh_all[:].rearrange("p (c s) -> p c s", s=S)

                # text pair tiles [128, 2, 1024]: partition = position within
                # chunk, dim1 = chunk parity (the DoubleRow k-tile pair)
                txts = []
                for c in range(NPAIR):
                    txc = text_pool.tile([128, 2, D], F8, tag="tx", name=f"tx_{b}_{c}")
                    rows = c * 256
                    nc.sync.dma_start(txc[:, 0, :], text_d[b, rows:rows + 128, :])
                    nc.sync.dma_start(txc[:, 1, :], text_d[b, rows + 128:rows + 256, :])
                    txts.append(txc)

                # ssT chains: one PSUM bank per d-tile, accumulated over the
                # 4 chunk pairs in DoubleRow mode (K=256 per matmul)
                psts = [
                    psum_pool.tile([128, S], F32, tag="ss", name=f"pst_{b}_{t}")
                    for t in range(NDT)
                ]
                for c in range(NPAIR):
                    ohr = oh_v[:, 2 * c:2 * c + 2, :]
                    for t in range(NDT):
                        nc.tensor.matmul(
                            psts[t][:],
                            txts[c][:, :, t * 128:(t + 1) * 128],
                            ohr,
                            start=(c == 0), stop=(c == NPAIR - 1),
                            perf_mode=PERF.DoubleRow,
                        )

                # copy ssT out of PSUM (split between DVE and ACT), then
                # prod = ssT*vgT (DVE) and ss^2 (ACT), all [128, 256]-shaped
                sst = sst_pool.tile([128, NDT, S], BF16, tag="sst", name=f"sst_{b}")
                for t in range(NDT):
                    if t % 2 == 0:
                        nc.vector.tensor_copy(sst[:, t, :], psts[t][:])
                    else:
                        nc.scalar.activation(sst[:, t, :], psts[t][:], ACTF.Copy)
                cb = combos[b]
                nc.vector.tensor_tensor(
                    cb[:, :, 0:S], sst[:],
                    vgt[:, b * NDT * S:(b + 1) * NDT * S]
                    .rearrange("p (t s) -> p t s", s=S),
                    op=ALU.mult,
                )
                nc.scalar.activation(cb[:, :, S:2 * S], sst[:], ACTF.Square)

                # partition-reduce (num | ss^2 | vg^2) over d via ones-matmul
                red = psumr_pool.tile([1, 3 * S], F32, tag="red", name=f"red_{b}")
                for t in range(NDT):
                    nc.tensor.matmul(
                        red[:], ones_bf[:], cb[:, t, :],
                        start=(t == 0), stop=(t == NDT - 1),
                    )
                nc.vector.tensor_copy(asm[:, b, :], red[:])

            # ---- batched cosine assembly over all (b, s) at [1, 256] ----
            num_v = asm[:, :, 0:S]
            nss_v = asm[:, :, S:2 * S]
            nvg_v = asm[:, :, 2 * S:3 * S]
            prodn = small_pool.tile([1, BPC, S], F32, name="prodn")
            nc.vector.tensor_tensor(prodn[:], nss_v, nvg_v, op=ALU.mult)
            sq = small_pool.tile([1, BPC, S], F32, name="sq")
            nc.scalar.sqrt(sq[:], prodn[:])
            den = small_pool.tile([1, BPC, S], F32, name="den")
            nc.vector.tensor_scalar(
                out=den[:], in0=sq[:], scalar1=float(EPS), scalar2=None,
                op0=ALU.max,
            )
            rec = small_pool.tile([1, BPC, S], F32, name="rec")
            nc.vector.reciprocal(rec[:], den[:])
            cos_all = small_pool.tile([1, BPC, S], F32, name="cos_all")
            nc.vector.tensor_tensor(cos_all[:], num_v, rec[:], op=ALU.mult)

            nc.sync.dma_start(out_d[:], cos_all[:].rearrange("o b s -> o (b s)"))

    nc.compile()
    return nc


def pack_shard(attributes, text_feats, Vgs):
    """Host-side packing of one core's shard into the kernel's dram layout."""
    # vgt[p, b, t, s] = Vgs[b, s, t*128 + p]
    vg = np.asarray(Vgs, dtype=np.float32).reshape(BPC, S, NDT, 128)
    vgt = np.ascontiguousarray(vg.transpose(3, 0, 2, 1)).reshape(128, BPC * NDT * S)
    return {
        "attributes": np.ascontiguousarray(attributes, dtype=np.int32),
        "text_feats": np.ascontiguousarray(
            np.asarray(text_feats, dtype=np.float32).astype(NP_F8)
        ),
        "Vgs": vgt.astype(NP_F8),
    }


_NC_CACHE = None


def _get_nc():
    global _NC_CACHE
    if _NC_CACHE is None:
        _NC_CACHE = build_bass()
    return _NC_CACHE


def kernel(attributes: np.ndarray, text_feats: np.ndarray, Vgs: np.ndarray) -> np.ndarray:
    assert attributes.shape == (B, L) and attributes.dtype == np.int32
    assert text_feats.shape == (B, L, D)
    assert Vgs.shape == (B, S, D)
    nc = _get_nc()
    in_maps = [
        pack_shard(
            attributes[i * BPC:(i + 1) * BPC],
            text_feats[i * BPC:(i + 1) * BPC],
            Vgs[i * BPC:(i + 1) * BPC],
        )
        for i in range(N_CORES)
    ]
    res = run_bass_kernel_spmd(nc, in_maps, core_ids=list(range(N_CORES)))
    total = sum(float(r["out"].sum()) for r in res.results)
    loss = 1.0 - total / (B * S)
    return np.asarray(loss, dtype=np.float32)


# revision 7
# speedup vs baseline: 3.0686x; 1.5825x over previous
"""Trainium2 Bass kernel for the segment-reduce cosine loss problem.

Reference computation (per sample b, S=32 labels):
  onehot[l,s] = (attributes[b,l] == s+1)
  seg_sum[s,:] = sum_l onehot[l,s] * text_feats[b,l,:]
  seg_mean     = seg_sum / count[s]
  cos[s] = <Vgs[b,s], seg_mean[s]> / max(|Vgs[b,s]| * |seg_mean[s]|, 1e-8)
  loss = mean_b (1 - mean_s cos[b,s]) = 1 - (sum_{b,s} cos) / (B*S)

Sharding: pure data parallel over batch; each of the 8 cores handles 8
samples and returns its 256 cos values; the host averages.

Performance design (cost model: DMA 360 GB/s aggregate, HWDGE 625 ns/DMA,
PE 2.4 GHz, fp8 DoubleRow 0.5 cyc/row):
  - text_feats is quantized to fp8-e4m3 on the host, cutting the dominant
    HBM stream from 32 MB to 8 MB per core (~24 us at 360 GB/s).  Cosine
    is scale-invariant in seg_sum and the loss averages 2048 cos values,
    so fp8 noise lands ~3e-5 relative on the loss (gate is 2e-2).
  - One DMA per sample: the host pre-packs each sample's text
    partition-major ([p, chunk-pair, parity, d] = 8448 B/partition incl.
    the sample's transposed Vgs block), so a sample is a single
    128-descriptor DMA.  This keeps the serial HWDGE descriptor-gen cost
    (625 ns/DMA) off the critical path (67 DMAs cost 42 us; 18 cost 11).
  - Segment sums run on the PE in fp8 DoubleRow mode: lhsT = text d-tile
    [128L, 2, 128D] (stationary), rhs = onehot pair [128L, 2, 32]
    (moving), K=256 per instruction at 0.5 cyc/row -> ssT [128D, 32S] per
    d-tile in PSUM, 16 cycles per matmul.
  - The transposed [D, S] layout puts the epilogue on all 128 partitions:
    Pool (idle otherwise) copies ssT banks to SBUF, DVE computes
    prod=ssT*vgT and ACT squares ssT/vgT as [128, 256] ops, and the
    D-reductions (num, |ss|^2, |Vg|^2) are a ones-vector matmul chain
    into PSUM [1, 96] per sample.
  - attributes are host-transposed to [p, b, chunk] int8 so onehot is a
    single is_equal per sample against an int8 iota (no PE transpose).
  - The last sample's text is packed d-tile-major and fetched as 8 DMAs
    so its chains + epilogue stagger with the stream and only the final
    d-tile's work sits after the last byte.
"""

import numpy as np
import ml_dtypes

import concourse.mybir as mybir
import concourse.tile as tile
from concourse import bacc
from concourse.bass_utils import run_bass_kernel_spmd

B, L, D, S = 64, 1024, 1024, 32
N_CORES = 8
BPC = B // N_CORES        # samples per core
NCHUNK = L // 128         # L-chunks of 128 positions
NPAIR = NCHUNK // 2       # DoubleRow chunk pairs (256 positions each)
NDT = D // 128            # d-tiles of 128 feature columns
EPS = 1e-8
TXT_B = NPAIR * 2 * D     # 8192 text bytes per partition per sample
ROW_B = TXT_B + NDT * S   # + 256 transposed-Vgs bytes

F32 = mybir.dt.float32
F8 = mybir.dt.float8e4
BF16 = mybir.dt.bfloat16
I8 = mybir.dt.int8
ALU = mybir.AluOpType
ACTF = mybir.ActivationFunctionType
PERF = mybir.MatmulPerfMode

NP_F8 = ml_dtypes.float8_e4m3


def build_bass():
    nc = bacc.Bacc(
        "TRN2", target_bir_lowering=False, debug=False, num_devices=N_CORES
    )
    attrs_d = nc.dram_tensor(
        "attributes", [128, BPC * NCHUNK], I8, kind="ExternalInput"
    )
    text_d = nc.dram_tensor("text_feats", [BPC, 128, ROW_B], F8, kind="ExternalInput")
    out_d = nc.dram_tensor("out", [1, BPC * S], F32, kind="ExternalOutput")

    with tile.TileContext(nc) as tc:
        with (
            tc.tile_pool(name="const", bufs=1) as const_pool,
            tc.tile_pool(name="text", bufs=4) as text_pool,
            tc.tile_pool(name="oh", bufs=4) as oh_pool,
            tc.tile_pool(name="sst", bufs=2) as sst_pool,
            tc.tile_pool(name="combo", bufs=BPC) as combo_pool,
            tc.tile_pool(name="small", bufs=2) as small_pool,
            tc.tile_pool(name="psum", bufs=7, space="PSUM") as psum_pool,
            tc.tile_pool(name="psumr", bufs=1, space="PSUM") as psumr_pool,
        ):
            # ---- constants / warms ----
            iota_s = const_pool.tile([128, S], I8, name="iota_s")
            nc.gpsimd.iota(
                iota_s[:], pattern=[[1, S]], base=1, channel_multiplier=0,
                allow_small_or_imprecise_dtypes=True,
            )
            warm = const_pool.tile([128, 1], F32, name="warm")
            nc.vector.memset(warm[:], 1.0)
            nc.scalar.activation(warm[:], warm[:], ACTF.Sqrt)
            nc.scalar.activation(warm[:], warm[:], ACTF.Square)
            ones_bf = const_pool.tile([128, 1], BF16, name="ones_bf")
            nc.vector.memset(ones_bf[:], 1.0)

            # sample 0 text first (stream start), then the tiny attr block
            txs = [None] * BPC
            txs[0] = text_pool.tile([128, ROW_B], F8, tag="tx", name="tx_0")
            nc.sync.dma_start(txs[0][:], text_d[0])
            attr_sb = const_pool.tile([128, BPC * NCHUNK], I8, name="attr_sb")
            nc.sync.dma_start(attr_sb[:], attrs_d[:])
            # last sample's Vgs block early so its |Vg|^2 is ready
            txs[7] = text_pool.tile([128, ROW_B], F8, tag="tx7", bufs=1, name="tx_7")
            nc.sync.dma_start(txs[7][:, TXT_B:ROW_B], text_d[BPC - 1, :, TXT_B:ROW_B])
            for b in range(1, BPC - 1):
                txs[b] = text_pool.tile([128, ROW_B], F8, tag="tx", name=f"tx_{b}")
                nc.sync.dma_start(txs[b][:], text_d[b])
            # last sample: d-tile-major packing, one DMA per d-tile
            for t in range(NDT):
                nc.sync.dma_start(
                    txs[7][:, t * 1024:(t + 1) * 1024],
                    text_d[BPC - 1, :, t * 1024:(t + 1) * 1024],
                )

            asm = const_pool.tile([1, BPC, 3 * S], F32, name="asm")
            cos_all = const_pool.tile([1, BPC, S], F32, name="cos_all")

            for b in range(BPC):
                last = b == BPC - 1
                tx = txs[b]
                vg_v = tx[:, TXT_B:ROW_B].rearrange("p (t s) -> p t s", s=S)

                # combo[b]: [128, t, (prod | ss^2 | vg^2)]
                cb = combo_pool.tile([128, NDT, 3 * S], BF16, tag="cb", name=f"cb_{b}")
                nc.scalar.activation(cb[:, :, 2 * S:3 * S], vg_v, ACTF.Square)

                # onehot for all chunks in one is_equal: [p, (c s)] fp8
                oh_all = oh_pool.tile([128, NCHUNK * S], F8, tag="oh", name=f"oh_{b}")
                nc.vector.tensor_tensor(
                    oh_all[:].rearrange("p (c s) -> p c s", s=S),
                    attr_sb[:, b * NCHUNK:(b + 1) * NCHUNK]
                    .unsqueeze(2).broadcast_to([128, NCHUNK, S]),
                    iota_s[:].unsqueeze(1).broadcast_to([128, NCHUNK, S]),
                    op=ALU.is_equal,
                )
                oh_v = oh_all[:].rearrange("p (c s) -> p c s", s=S)

                psts = [
                    psum_pool.tile([128, S], F32, tag="ss", name=f"pst_{b}_{t}")
                    for t in range(NDT)
                ]
                if not last:
                    # pair-major packing: [p, c, i, d]
                    tx_v = tx[:, 0:TXT_B].rearrange(
                        "p (c i d) -> p c i d", c=NPAIR, i=2
                    )
                    for c in range(NPAIR):
                        ohr = oh_v[:, 2 * c:2 * c + 2, :]
                        for t in range(NDT):
                            nc.tensor.matmul(
                                psts[t][:],
                                tx_v[:, c, :, t * 128:(t + 1) * 128],
                                ohr,
                                start=(c == 0), stop=(c == NPAIR - 1),
                                perf_mode=PERF.DoubleRow,
                            )
                    # ssT -> SBUF (GPSIMD cannot read PSUM, so split the
                    # copies between DVE and ACT), then batched [128, 256]
                    # prod (DVE) and ss^2 (ACT)
                    sst = sst_pool.tile([128, NDT, S], BF16, tag="sst", name=f"sst_{b}")
                    for t in range(NDT):
                        if t % 2 == 0:
                            nc.vector.tensor_copy(sst[:, t, :], psts[t][:])
                        else:
                            nc.scalar.activation(sst[:, t, :], psts[t][:], ACTF.Copy)
                    nc.vector.tensor_tensor(
                        cb[:, :, 0:S], sst[:], vg_v, op=ALU.mult
                    )
                    nc.scalar.activation(cb[:, :, S:2 * S], sst[:], ACTF.Square)
                else:
                    # d-tile-major packing: [p, t, c, i, ds]; per-t epilogue
                    # staggers with the 8 d-tile DMAs
                    tx_v = tx[:, 0:TXT_B].rearrange(
                        "p (t c i e) -> p t c i e", t=NDT, c=NPAIR, i=2
                    )
                    for t in range(NDT):
                        for c in range(NPAIR):
                            nc.tensor.matmul(
                                psts[t][:],
                                tx_v[:, t, c, :, :],
                                oh_v[:, 2 * c:2 * c + 2, :],
                                start=(c == 0), stop=(c == NPAIR - 1),
                                perf_mode=PERF.DoubleRow,
                            )
                    for t in range(NDT):
                        nc.vector.tensor_tensor(
                            cb[:, t, 0:S], psts[t][:], vg_v[:, t, :], op=ALU.mult
                        )
                        nc.scalar.activation(cb[:, t, S:2 * S], psts[t][:], ACTF.Square)

                # partition-reduce (num | ss^2 | vg^2) over d via ones-matmul
                red = psumr_pool.tile([1, 3 * S], F32, tag="red", name=f"red_{b}")
                for t in range(NDT):
                    nc.tensor.matmul(
                        red[:], ones_bf[:], cb[:, t, :],
                        start=(t == 0), stop=(t == NDT - 1),
                    )
                nc.vector.tensor_copy(asm[:, b, :], red[:])

            # ---- cosine assembly: batch samples 0..6, then sample 7 ----
            # cos = num / max(sqrt(|ss|^2 * |vg|^2), EPS)
            for lo, hi in ((0, BPC - 1), (BPC - 1, BPC)):
                n = hi - lo
                num_v = asm[:, lo:hi, 0:S]
                nss_v = asm[:, lo:hi, S:2 * S]
                nvg_v = asm[:, lo:hi, 2 * S:3 * S]
                pr = small_pool.tile([1, n, S], F32, tag=f"pr{lo}", name=f"pr_{lo}")
                nc.vector.tensor_tensor(pr[:], nss_v, nvg_v, op=ALU.mult)
                sq = small_pool.tile([1, n, S], F32, tag=f"sq{lo}", name=f"sq_{lo}")
                nc.scalar.sqrt(sq[:], pr[:])
                dn = small_pool.tile([1, n, S], F32, tag=f"dn{lo}", name=f"dn_{lo}")
                nc.vector.tensor_scalar(
                    out=dn[:], in0=sq[:], scalar1=float(EPS), scalar2=None,
                    op0=ALU.max,
                )
                rs = small_pool.tile([1, n, S], F32, tag=f"rs{lo}", name=f"rs_{lo}")
                nc.vector.reciprocal(rs[:], dn[:])
                nc.vector.tensor_tensor(
                    cos_all[:, lo:hi, :], num_v, rs[:], op=ALU.mult
                )

            nc.sync.dma_start(out_d[:], cos_all[:].rearrange("o b s -> o (b s)"))

    nc.compile()
    return nc


def pack_shard(attributes, text_feats, Vgs):
    """Host-side packing of one core's shard into the kernel's dram layout."""
    at = np.asarray(attributes)
    # attr[p, b, c] = attributes[b, c*128 + p], int8 (values 0..32)
    attr_tp = np.ascontiguousarray(
        at.reshape(BPC, NCHUNK, 128).transpose(2, 0, 1).reshape(128, BPC * NCHUNK)
    ).astype(np.int8)

    tf8 = np.asarray(text_feats, dtype=np.float32).astype(NP_F8)
    vg8 = np.asarray(Vgs, dtype=np.float32).astype(NP_F8)
    t8 = np.empty((BPC, 128, ROW_B), dtype=NP_F8)
    x = tf8.reshape(BPC, NPAIR, 2, 128, D)
    for b in range(BPC - 1):
        # [p, c, i, d]
        t8[b, :, 0:TXT_B] = x[b].transpose(2, 0, 1, 3).reshape(128, TXT_B)
    # last sample: [p, t, c, i, ds]
    x7 = x[BPC - 1].reshape(NPAIR, 2, 128, NDT, 128)
    t8[BPC - 1, :, 0:TXT_B] = x7.transpose(2, 3, 0, 1, 4).reshape(128, TXT_B)
    # vgt tail: [p, t, s] = Vgs[b, s, t*128+p]
    vgt = vg8.reshape(BPC, S, NDT, 128).transpose(0, 3, 2, 1)
    t8[:, :, TXT_B:ROW_B] = vgt.reshape(BPC, 128, NDT * S)
    return {"attributes": attr_tp, "text_feats": t8}


_NC_CACHE = None


def _get_nc():
    global _NC_CACHE
    if _NC_CACHE is None:
        _NC_CACHE = build_bass()
    return _NC_CACHE


def kernel(attributes: np.ndarray, text_feats: np.ndarray, Vgs: np.ndarray) -> np.ndarray:
    assert attributes.shape == (B, L) and attributes.dtype == np.int32
    assert text_feats.shape == (B, L, D)
    assert Vgs.shape == (B, S, D)
    nc = _get_nc()
    in_maps = [
        pack_shard(
            attributes[i * BPC:(i + 1) * BPC],
            text_feats[i * BPC:(i + 1) * BPC],
            Vgs[i * BPC:(i + 1) * BPC],
        )
        for i in range(N_CORES)
    ]
    res = run_bass_kernel_spmd(nc, in_maps, core_ids=list(range(N_CORES)))
    total = sum(float(r["out"].sum()) for r in res.results)
    loss = 1.0 - total / (B * S)
    return np.asarray(loss, dtype=np.float32)
